# revision 8
# baseline (speedup 1.0000x reference)
"""Trainium2 Bass kernel for nn_MultiHeadedAttention (dense transformer block).

Reference computation (per batch b of 2):
  x   = pad(inp, 2 left / 2 right on feature dim)          [2048, 512]
  q/k/v = x @ W{q,k,v}.T, split into 8 heads of 64         [8, 2048, 64]
  attn = softmax(q @ k^T / 8)                              [8, 2048, 2048]  (output!)
  out  = attn @ v, concat heads, + residual, layernorm     [2048, 512] -> strip pad
Returns (out[2,2048,508], attn[2,8,2048,2048]).

Sharding: 8 cores, core c handles batch c//4, sequence rows (c%4)*512..+512,
all 8 heads. Fully SPMD, no collectives. Each core:
  - builds x^T via PE transposes, projects K^T = Wk @ x^T, V = x @ Wv^T,
    Q^T = Wq @ xs^T (f32r matmuls, 1 cycle/row)
  - per head pair: scores^T -> exp (one ACT pass, scale=1/8 folded in) -> A@V
  - per head: natural-layout scores -> exp (+fused rowsum) -> normalize -> HBM
  - out^T transposed back on PE, per-head softmax denominators applied,
    residual + layernorm on DVE/ACT.
"""

import sys

sys.path.insert(0, "/opt/trn_rl_repo")

import numpy as np

L = 2048          # sequence length
LS = 512          # per-core sequence slice
D = 512           # padded feature dim
H = 8             # heads
DH = 64           # head dim
P = 128           # partitions
N_CORES = 8
EPS = 1e-6

_COMPILED = {}


def _build(reps=1):
    import concourse.bass as bass  # noqa: F401
    import concourse.tile as tile
    import concourse.mybir as mybir
    from concourse import bacc
    from concourse.masks import make_identity

    dt = mybir.dt
    AF = mybir.ActivationFunctionType
    ALU = mybir.AluOpType

    nc = bacc.Bacc(None)

    x_d = nc.dram_tensor("x", [L, D], dt.float32, kind="ExternalInput")
    xs_d = nc.dram_tensor("xs", [LS, D], dt.float32, kind="ExternalInput")
    wqt_d = nc.dram_tensor("wqt", [D, D], dt.float32, kind="ExternalInput")
    wkt_d = nc.dram_tensor("wkt", [D, D], dt.float32, kind="ExternalInput")
    wvt_d = nc.dram_tensor("wvt", [D, D], dt.float32, kind="ExternalInput")
    gamma_d = nc.dram_tensor("gamma", [D], dt.float32, kind="ExternalInput")
    beta_d = nc.dram_tensor("beta", [D], dt.float32, kind="ExternalInput")
    attn_d = nc.dram_tensor("attn_o", [H, LS, L], dt.float32, kind="ExternalOutput")
    out_d = nc.dram_tensor("out_o", [LS, D], dt.float32, kind="ExternalOutput")

    MT = L // P       # 16 m tiles
    LT = LS // P      # 4 l tiles
    CT = D // P       # 4 contraction tiles
    NB = 512          # matmul free-dim chunk

    with tile.TileContext(nc) as tc:
        with (
            tc.tile_pool(name="singles", bufs=1) as singles,
            tc.tile_pool(name="epool", bufs=2) as epool,
            tc.tile_pool(name="etpool", bufs=2) as etpool,
            tc.tile_pool(name="ln", bufs=2) as lnp,
            tc.tile_pool(name="pp_sm", bufs=2, space="PSUM") as pp_sm,
            tc.tile_pool(name="pp_st", bufs=1, space="PSUM") as pp_st,
            tc.tile_pool(name="pp_nat", bufs=1, space="PSUM") as pp_nat,
        ):
            ident = singles.tile([P, P], dt.float32, tag="ident")
            make_identity(nc, ident[:])
            gamma_b = singles.tile([P, D], dt.float32, tag="gamma_b")
            nc.sync.dma_start(out=gamma_b[:], in_=gamma_d[:].partition_broadcast(P))
            beta_b = singles.tile([P, D], dt.float32, tag="beta_b")
            nc.sync.dma_start(out=beta_b[:], in_=beta_d[:].partition_broadcast(P))
            eps_t = singles.tile([P, 1], dt.float32, tag="eps")
            nc.vector.memset(eps_t[:], EPS)

            KT = singles.tile([P, CT, L], dt.float32r, tag="KT")
            QT = singles.tile([P, CT, LS], dt.float32r, tag="QT")
            V = singles.tile([P, MT, D], dt.float32r, tag="V")
            xs_nat = singles.tile([P, LT, D], dt.float32, tag="xs_nat")
            outT_sb = singles.tile([DH, H, LS], dt.float32, tag="outT_sb")
            y_nat = singles.tile([P, LT, D], dt.float32, tag="y_nat")
            rec_all = singles.tile([P, H, LT], dt.float32, tag="rec_all")

            import contextlib

            if isinstance(reps, int) and reps > 1:
                rep_ctx = lambda: tc.For_i(0, reps, 1)  # noqa: E731
                rep_iter = [0]
            else:
                rep_ctx = contextlib.nullcontext
                rep_iter = [0]

            with rep_ctx() as _i:
              for _rep in rep_iter:
                # ---------- load, transpose x, projections (scoped: freed after) ----------
                with tc.tile_pool(name="proj", bufs=1) as projp, \
                     tc.tile_pool(name="wpool", bufs=2) as wpool, \
                     tc.tile_pool(name="xnp", bufs=2) as xnp:
                    xT = projp.tile([P, CT, L], dt.float32r, tag="xT")
                    xsT = projp.tile([P, CT, LS], dt.float32r, tag="xsT")

                    for mt in range(MT):
                        xn = xnp.tile([P, D], dt.float32, tag="xn")
                        nc.sync.dma_start(out=xn[:], in_=x_d[mt * P:(mt + 1) * P, :])
                        pp = pp_sm.tile([P, NB], dt.float32, tag="ps")
                        for ct in range(CT):
                            nc.tensor.transpose(
                                pp[:, ct * P:(ct + 1) * P], xn[:, ct * P:(ct + 1) * P],
                                ident[:])
                        nc.vector.tensor_copy(
                            xT[:, :, mt * P:(mt + 1) * P].bitcast(dt.float32r), pp[:])

                    for lt in range(LT):
                        nc.sync.dma_start(
                            out=xs_nat[:, lt, :], in_=xs_d[lt * P:(lt + 1) * P, :])
                    for lt in range(LT):
                        pp = pp_sm.tile([P, NB], dt.float32, tag="ps")
                        for ct in range(CT):
                            nc.tensor.transpose(
                                pp[:, ct * P:(ct + 1) * P],
                                xs_nat[:, lt, ct * P:(ct + 1) * P], ident[:])
                        nc.vector.tensor_copy(
                            xsT[:, :, lt * P:(lt + 1) * P].bitcast(dt.float32r), pp[:])

                    # QT = Wq @ xs^T
                    wq = wpool.tile([P, CT, D], dt.float32r, tag="w")
                    nc.sync.dma_start(
                        out=wq[:],
                        in_=wqt_d[:].rearrange("(t p) d -> p t d", p=P).bitcast(dt.float32r))
                    for dt_ in range(CT):
                        pp = pp_sm.tile([P, NB], dt.float32, tag="ps")
                        for ct in range(CT):
                            nc.tensor.matmul(
                                pp[:],
                                wq[:, ct, dt_ * P:(dt_ + 1) * P],
                                xsT[:, ct, :],
                                start=(ct == 0), stop=(ct == CT - 1))
                        nc.vector.tensor_copy(QT[:, dt_, :].bitcast(dt.float32r), pp[:])
                    # KT[dt_] rows = (Wk @ x^T)[dt_*128..]
                    wk = wpool.tile([P, CT, D], dt.float32r, tag="w")
                    nc.sync.dma_start(
                        out=wk[:],
                        in_=wkt_d[:].rearrange("(t p) d -> p t d", p=P).bitcast(dt.float32r))
                    for dt_ in range(CT):
                        for mc in range(L // NB):
                            pp = pp_sm.tile([P, NB], dt.float32, tag="ps")
                            for ct in range(CT):
                                nc.tensor.matmul(
                                    pp[:],
                                    wk[:, ct, dt_ * P:(dt_ + 1) * P],
                                    xT[:, ct, mc * NB:(mc + 1) * NB],
                                    start=(ct == 0), stop=(ct == CT - 1))
                            nc.vector.tensor_copy(
                                KT[:, dt_, mc * NB:(mc + 1) * NB].bitcast(dt.float32r),
                                pp[:])
                    # V rows = x @ Wv^T
                    wv = wpool.tile([P, CT, D], dt.float32r, tag="w")
                    nc.sync.dma_start(
                        out=wv[:],
                        in_=wvt_d[:].rearrange("(t p) d -> p t d", p=P).bitcast(dt.float32r))
                    for mt in range(MT):
                        pp = pp_sm.tile([P, NB], dt.float32, tag="ps")
                        for ct in range(CT):
                            nc.tensor.matmul(
                                pp[:],
                                xT[:, ct, mt * P:(mt + 1) * P],
                                wv[:, ct, :],
                                start=(ct == 0), stop=(ct == CT - 1))
                        nc.vector.tensor_copy(V[:, mt, :].bitcast(dt.float32r), pp[:])

                # ---------- attention ----------
                for hp in range(H // 2):          # head pairs (2*hp, 2*hp+1)
                    h0 = 2 * hp
                    # S^T -> exp -> A@V for the pair
                    av0 = pp_sm.tile([DH, NB], dt.float32, tag="ps")
                    av1 = pp_sm.tile([DH, NB], dt.float32, tag="ps")
                    avs = (av0, av1)
                    for mt in range(MT):
                        st = pp_st.tile([P, 2 * LS], dt.float32, tag="st")
                        for j in range(2):
                            h = h0 + j
                            dt_, rp = h // 2, (h % 2) * DH
                            nc.tensor.matmul(
                                st[:, j * LS:(j + 1) * LS],
                                KT[rp:rp + DH, dt_, mt * P:(mt + 1) * P],
                                QT[rp:rp + DH, dt_, :],
                                start=True, stop=True)
                        et = etpool.tile([P, 2 * LS], dt.float32r, tag="et")
                        nc.scalar.activation(out=et[:], in_=st[:], func=AF.Exp, scale=0.125)
                        for j in range(2):
                            h = h0 + j
                            nc.tensor.matmul(
                                avs[j][:],
                                V[:, mt, h * DH:(h + 1) * DH],
                                et[:, j * LS:(j + 1) * LS],
                                start=(mt == 0), stop=(mt == MT - 1))
                    # evacuate av pair -> outT_sb (unnormalized)
                    for j in range(2):
                        h = h0 + j
                        nc.vector.tensor_copy(outT_sb[:, h, :], avs[j][:])

                    # natural-layout scores + softmax + attn DMA, per head / l-tile
                    for j in range(2):
                        h = h0 + j
                        dt_, rp = h // 2, (h % 2) * DH
                        for lt in range(LT):
                            sn = pp_nat.tile([P, L], dt.float32, tag="sn")
                            for mc in range(L // NB):
                                nc.tensor.matmul(
                                    sn[:, mc * NB:(mc + 1) * NB],
                                    QT[rp:rp + DH, dt_, lt * P:(lt + 1) * P],
                                    KT[rp:rp + DH, dt_, mc * NB:(mc + 1) * NB],
                                    start=True, stop=True)
                            e = epool.tile([P, L], dt.float32, tag="e")
                            rs = lnp.tile([P, 1], dt.float32, tag="rs")
                            nc.scalar.activation(
                                out=e[:], in_=sn[:], func=AF.Exp, scale=0.125,
                                accum_out=rs[:])
                            nc.vector.reciprocal(
                                out=rec_all[:, h, lt:lt + 1], in_=rs[:])
                            nc.vector.tensor_scalar_mul(
                                e[:], e[:], rec_all[:, h, lt:lt + 1])
                            nc.sync.dma_start(
                                out=attn_d[h, lt * P:(lt + 1) * P, :], in_=e[:])

                # ---------- out^T -> out, normalize, residual + LN ----------
                for lt in range(LT):
                    po = pp_sm.tile([P, NB], dt.float32, tag="ps")
                    for h in range(H):
                        nc.tensor.transpose(
                            po[:, h * DH:(h + 1) * DH],
                            outT_sb[:, h, lt * P:(lt + 1) * P],
                            ident[:DH, :DH])
                    for h in range(H):
                        nc.vector.tensor_scalar_mul(
                            y_nat[:, lt, h * DH:(h + 1) * DH],
                            po[:, h * DH:(h + 1) * DH],
                            rec_all[:, h, lt:lt + 1])

                    y = lnp.tile([P, D], dt.float32, tag="y")
                    nc.vector.tensor_add(y[:], y_nat[:, lt, :], xs_nat[:, lt, :])
                    st6 = lnp.tile([P, 6], dt.float32, tag="st6")
                    nc.vector.bn_stats(out=st6[:], in_=y[:])
                    mv = lnp.tile([P, 2], dt.float32, tag="mv")
                    nc.vector.bn_aggr(out=mv[:], in_=st6[:])
                    sd = lnp.tile([P, 1], dt.float32, tag="sd")
                    nc.scalar.activation(
                        out=sd[:], in_=mv[:, 1:2], func=AF.Sqrt, bias=eps_t[:], scale=1.0)
                    rstd = lnp.tile([P, 1], dt.float32, tag="rstd")
                    nc.vector.reciprocal(out=rstd[:], in_=sd[:])
                    z = lnp.tile([P, D], dt.float32, tag="z")
                    nc.vector.tensor_scalar(
                        out=z[:], in0=y[:], scalar1=mv[:, 0:1], scalar2=rstd[:],
                        op0=ALU.subtract, op1=ALU.mult)
                    nc.vector.tensor_mul(z[:], z[:], gamma_b[:])
                    nc.vector.tensor_add(z[:], z[:], beta_b[:])
                    nc.sync.dma_start(out=out_d[lt * P:(lt + 1) * P, :], in_=z[:])

    nc.finalize()
    return nc


def _get_compiled(reps=1):
    if reps not in _COMPILED:
        _COMPILED[reps] = _build(reps)
    return _COMPILED[reps]


def kernel(inp, Wq, Wk, Wv, gamma, beta, reps=1, _return_results=False):
    from concourse.bass_utils import run_bass_kernel_spmd

    inp = np.asarray(inp, dtype=np.float32)
    B = inp.shape[0]
    x_pad = np.pad(inp, ((0, 0), (0, 0), (2, 2)))      # [2, 2048, 512]
    wqt = np.ascontiguousarray(np.asarray(Wq, np.float32).T)
    wkt = np.ascontiguousarray(np.asarray(Wk, np.float32).T)
    wvt = np.ascontiguousarray(np.asarray(Wv, np.float32).T)
    gamma = np.ascontiguousarray(np.asarray(gamma, np.float32))
    beta = np.ascontiguousarray(np.asarray(beta, np.float32))

    in_maps = []
    for c in range(N_CORES):
        b, sl = c // 4, c % 4
        in_maps.append({
            "x": np.ascontiguousarray(x_pad[b]),
            "xs": np.ascontiguousarray(x_pad[b, sl * LS:(sl + 1) * LS, :]),
            "wqt": wqt, "wkt": wkt, "wvt": wvt,
            "gamma": gamma, "beta": beta,
        })

    nc = _get_compiled(reps)
    res = run_bass_kernel_spmd(nc, in_maps, core_ids=list(range(N_CORES)))

    attn = np.empty((B, H, L, L), dtype=np.float32)
    out = np.empty((B, L, 508), dtype=np.float32)
    for c in range(N_CORES):
        b, sl = c // 4, c % 4
        r = res.results[c]
        attn[b, :, sl * LS:(sl + 1) * LS, :] = r["attn_o"]
        out[b, sl * LS:(sl + 1) * LS, :] = r["out_o"][:, 2:510]
    if _return_results:
        return (out, attn), res
    return out, attn
